# revision 1
# baseline (speedup 1.0000x reference)
"""Multi-level (FPN) DeformRoIPool (zero-offset == aligned RoIAlign) for Trainium2.

Strategy (8 NeuronCores, SPMD, one Bass program):
- Shard the 256 ROIs across cores (32 each); feature maps are preprocessed on
  host into per-ROI gather windows (channels-last pair-rows), so each core only
  uploads/reads the rows its ROIs touch.
- Per sample point (7x7 bins x 2x2 samples = 196 per ROI) one dma_gather
  element of 4KB covers the whole 2x2 bilinear patch: the window stores row
  pairs [F(y), F(y+1 clamped)] per (y, x) position (512 f32), and the gather
  element spans two consecutive x positions (1024 f32, overlapping stride).
- The weighted reduction over (sample, corner) -> (bin) runs on the PE as
  small matmuls with a host-built sparse weight matrix per ROI, accumulating
  in PSUM [49 bins, 256 c]. Host transposes [roi, bin, c] -> [roi, c, 7, 7].
"""
import os
import sys
import types

import numpy as np

OUT = 7
SR = 2
STRIDES = (4, 8, 16, 32)
FINEST = 56.0
IMG = 800.0
NLEV = 4
C = 256
N_ROIS = 256
N_CORES = 8
NROI_C = N_ROIS // N_CORES          # 32 rois per core
ROIS_PER_CALL = 4
NCALL = NROI_C // ROIS_PER_CALL     # gather calls per core
NSAMP = OUT * OUT * SR * SR         # 196 samples per roi
NREAL_CALL = ROIS_PER_CALL * NSAMP  # real gather idxs per call
NI_CALL = -(-NREAL_CALL // 16) * 16  # padded to x16 with trailing -1 (skipped)
NGRP_CALL = -(-NREAL_CALL // 128)   # slot groups per call
# flat (group, roi) matmul sets
GROUP_SETS = []
GROUP_K = []
for _g in range(NGRP_CALL):
    _lo, _hi = _g * 128, min((_g + 1) * 128, NREAL_CALL)
    GROUP_K.append(_hi - _lo)
    for _j in range(_lo // NSAMP, (_hi - 1) // NSAMP + 1):
        GROUP_SETS.append((_g, _j))
NSETS = len(GROUP_SETS)
WIN_R = 14 * 200                    # pair-row positions reserved per roi (l0 worst case)
WIN_STRIDE = WIN_R + 1              # +1 guard row per roi block
FEAT_SHAPES = [(2, 256, 200, 200), (2, 256, 100, 100), (2, 256, 50, 50), (2, 256, 25, 25)]


# ---------------------------------------------------------------------------
# BIR fix: this container's walrus rejects >1 embedded sem wait per
# instruction (2 on EventSemaphore). Split excess waits onto EventSemaphore
# carriers at serialization time.
# ---------------------------------------------------------------------------
def _install_bir_waitsplit():
    import orjson
    import concourse.bass as bass

    if getattr(bass.Bass, "_waitsplit_patched", False):
        return

    def _fix_blocks(blocks, counter):
        for blk in blocks:
            insts = blk.get("instructions")
            if insts:
                out = []
                for ins in insts:
                    si = ins.get("sync_info")
                    ow = (si or {}).get("on_wait") or []
                    limit = 2 if ins.get("opcode") == "EventSemaphore" else 1
                    if len(ow) > limit:
                        excess = ow[: len(ow) - limit]
                        si["on_wait"] = ow[len(ow) - limit:]
                        for i in range(0, len(excess), 2):
                            counter[0] += 1
                            out.append({
                                "name": f"I-waitsplit-{counter[0]}",
                                "opcode": "EventSemaphore",
                                "engine": ins["engine"],
                                "ins": [], "outs": [],
                                "debug": ins.get("debug", 0),
                                "sync_info": {"on_update": [], "on_wait": excess[i:i + 2]},
                            })
                    out.append(ins)
                blk["instructions"] = out
            if blk.get("blocks"):
                _fix_blocks(blk["blocks"], counter)

    orig = bass.Bass.to_json_bytes

    def to_json_bytes(self, *a, **kw):
        data = orig(self, *a, **kw)
        d = orjson.loads(data)
        counter = [0]
        for fn in d.get("functions", []):
            _fix_blocks(fn.get("blocks", []), counter)
        return orjson.dumps(d) if counter[0] else data

    bass.Bass.to_json_bytes = to_json_bytes
    bass.Bass._waitsplit_patched = True


# ---------------------------------------------------------------------------
# Host-side index / weight / window computation
# ---------------------------------------------------------------------------
def _roi_meta(rois):
    """Per-roi level + sample-grid floors and weights.

    Returns list of dicts with level l, batch b, and per-(i,si)/(j,sj) arrays.
    """
    scale_wh = np.sqrt((rois[:, 3] - rois[:, 1]) * (rois[:, 4] - rois[:, 2]))
    with np.errstate(divide="ignore"):
        tl = np.clip(np.floor(np.log2(scale_wh / FINEST + 1e-6)), 0, NLEV - 1)
    tl = (tl + 1e-5).astype(np.int32)
    g = np.arange(OUT, dtype=np.float64)[:, None] + (np.arange(SR, dtype=np.float64)[None, :] + 0.5) / SR
    metas = []
    for n in range(rois.shape[0]):
        l = int(tl[n])
        B, C_, H, W = FEAT_SHAPES[l]
        sc = 1.0 / STRIDES[l]
        x1 = rois[n, 1] * sc - 0.5
        y1 = rois[n, 2] * sc - 0.5
        rw = rois[n, 3] * sc - 0.5 - x1
        rh = rois[n, 4] * sc - 0.5 - y1
        y = y1 + (rh / OUT) * g  # [OUT, SR] sample y per (i, si)
        x = x1 + (rw / OUT) * g
        vy = (y > -1) & (y < H)
        vx = (x > -1) & (x < W)
        yc = np.clip(y, 0.0, H - 1)
        xc = np.clip(x, 0.0, W - 1)
        y0 = np.minimum(np.floor(yc).astype(np.int64), H - 1)
        x0 = np.minimum(np.floor(xc).astype(np.int64), W - 1)
        metas.append(dict(
            l=l, b=int(rois[n, 0]), H=H, W=W,
            y0=y0, x0=x0, ly=yc - y0, lx=xc - x0, vy=vy, vx=vx,
        ))
    return metas


def _build_core_inputs(feats_T, metas, core_rois):
    """Build win/idx/W tensors for one core's roi list (indices into metas)."""
    win = np.zeros((NROI_C * WIN_STRIDE + 1, 2 * C), np.float32)
    idx_all = np.full((NCALL, NI_CALL), -1, np.int16)
    wmat = np.zeros((NCALL, NSETS, 128, 4 * 49), np.float32)
    set_of = {(g_, j_): si_ for si_, (g_, j_) in enumerate(GROUP_SETS)}

    for rslot, n in enumerate(core_rois):
        m = metas[n]
        H, W = m["H"], m["W"]
        fT = feats_T[m["l"]][m["b"]]  # [H, W, C] channels-last view
        ys, yrank_of = np.unique(m["y0"]), {}
        for k, yv in enumerate(ys):
            yrank_of[yv] = k
        ysp1 = np.minimum(ys + 1, H - 1)
        nY = len(ys)
        # window block: rows [k*W + x] = [F(ys[k], x, :) | F(ys[k]+1c, x, :)]
        base = rslot * WIN_STRIDE
        blk = win[base:base + nY * W].reshape(nY, W, 2 * C)
        blk[:, :, :C] = fT[ys]
        blk[:, :, C:] = fT[ysp1]

        call, j = rslot // ROIS_PER_CALL, rslot % ROIS_PER_CALL
        jbase = j * WIN_STRIDE  # idx base within the call's 4-roi window span
        y0, x0, ly, lx = m["y0"], m["x0"], m["ly"], m["lx"]
        vy, vx = m["vy"], m["vx"]
        for i in range(OUT):
            for jj in range(OUT):
                for si in range(SR):
                    for sj in range(SR):
                        s = ((i * OUT + jj) * 4) + si * 2 + sj
                        slot = j * NSAMP + s
                        g_, p_ = slot // 128, slot % 128
                        yy0 = y0[i, si]
                        xx0 = x0[jj, sj]
                        idx_all[call, slot] = jbase + yrank_of[yy0] * W + xx0
                        v = (vy[i, si] and vx[jj, sj]) / (SR * SR)
                        hy = (1.0 - ly[i, si]) * v
                        lyv = ly[i, si] * v
                        hx = 1.0 - lx[jj, sj]
                        lxv = lx[jj, sj]
                        w0, w1, w2, w3 = hy * hx, lyv * hx, hy * lxv, lyv * lxv
                        if xx0 == W - 1:  # x1 clamps onto x0
                            w0, w2 = w0 + w2, 0.0
                            w1, w3 = w1 + w3, 0.0
                        b = s // 4
                        si_ = set_of[(g_, j)]
                        for q, w in enumerate((w0, w1, w2, w3)):
                            wmat[call, si_, p_, q * 49 + b] = w

    # idx layout per call: [128, NI/16], slot i -> [i%16, i//16], replicated x8
    idx_tiles = np.zeros((128, NCALL * (NI_CALL // 16)), np.int16)
    for c in range(NCALL):
        blk16 = idx_all[c].reshape(NI_CALL // 16, 16).T
        idx_tiles[:, c * (NI_CALL // 16):(c + 1) * (NI_CALL // 16)] = np.tile(blk16, (8, 1))
    return win, idx_tiles, wmat


def _build_core_inputs_fp16(feats_T, metas, core_rois):
    win, idx_tiles, wmat = _build_core_inputs(feats_T, metas, core_rois)
    return win.astype(np.float16), idx_tiles, wmat.astype(np.float16)


def _build_program():
    import concourse.bacc as bacc
    import concourse.mybir as mybir
    import concourse.tile as tile

    _install_bir_waitsplit()
    nc = bacc.Bacc("TRN2", debug=False, enable_asserts=True, num_devices=N_CORES)
    import concourse.bass as bass

    win_rows = NROI_C * WIN_STRIDE + 1
    win_d = nc.dram_tensor("win", [win_rows, 2 * C], mybir.dt.float16, kind="ExternalInput")
    idx_d = nc.dram_tensor("idx", [128, NCALL * (NI_CALL // 16)], mybir.dt.int16, kind="ExternalInput")
    w_d = nc.dram_tensor("wts", [NCALL * NSETS, 128, 4 * 49], mybir.dt.float16, kind="ExternalInput")
    out_d = nc.dram_tensor("out", [NROI_C, 49 * C], mybir.dt.float16, kind="ExternalOutput")


    with tile.TileContext(nc) as tc:
        with (
            tc.tile_pool(name="ip", bufs=1) as ip,
            tc.tile_pool(name="gp", bufs=8) as gp,
            tc.tile_pool(name="sp", bufs=3) as sp,
            tc.tile_pool(name="pp", bufs=8, space="PSUM") as pp,
        ):
            idx_t = ip.tile([128, NCALL * (NI_CALL // 16)], mybir.dt.int16)
            nc.sync.dma_start(idx_t[:], idx_d[:])
            wt = ip.tile([128, NCALL * NSETS * 4 * 49], mybir.dt.float16)
            nc.sync.dma_start(
                wt[:].rearrange("p (r w) -> p r w", w=4 * 49),
                w_d[:].rearrange("r p w -> p r w"),
            )
            for call in range(NCALL):
                g = gp.tile([128, NGRP_CALL * 4 * C], mybir.dt.float16, tag="g")
                # overlapping 4KB elems: row step 512 f32, elem 1024 f32
                src = bass.AP(
                    win_d[:].tensor,
                    call * ROIS_PER_CALL * WIN_STRIDE * (2 * C),
                    [[2 * C, ROIS_PER_CALL * WIN_STRIDE], [1, 4 * C]],
                )
                nc.gpsimd.dma_gather(
                    out_ap=g[:].rearrange("p (k c) -> p k c", c=4 * C),
                    in_ap=src,
                    idxs_ap=idx_t[:, call * (NI_CALL // 16):(call + 1) * (NI_CALL // 16)],
                    num_idxs=NI_CALL,
                    num_idxs_reg=NREAL_CALL,
                    elem_size=4 * C,
                    elem_step=2 * C,
                    single_packet=False,
                )
                st = sp.tile([49, ROIS_PER_CALL * C], mybir.dt.float16, tag="st")
                # first/last set index per roi j for start/stop flags
                firsts, lasts = {}, {}
                for si_, (g_, j_) in enumerate(GROUP_SETS):
                    firsts.setdefault(j_, si_)
                    lasts[j_] = si_
                ps_of = {j_: pp.tile([49, C], mybir.dt.float32, tag="ps", name=f"ps_{call}_{j_}") for j_ in range(ROIS_PER_CALL)}
                for si_, (g_, j_) in enumerate(GROUP_SETS):
                    K = GROUP_K[g_]
                    ps = ps_of[j_]
                    wb = (call * NSETS + si_) * 4 * 49
                    for q in range(4):
                        nc.tensor.matmul(
                            out=ps[:, :],
                            lhsT=wt[0:K, wb + q * 49:wb + (q + 1) * 49],
                            rhs=g[0:K, g_ * 4 * C + q * C:g_ * 4 * C + (q + 1) * C],
                            start=(si_ == firsts[j_] and q == 0),
                            stop=(si_ == lasts[j_] and q == 3),
                        )
                for j_ in range(ROIS_PER_CALL):
                    nc.vector.tensor_copy(st[:, j_ * C:(j_ + 1) * C], ps_of[j_][:])
                nc.sync.dma_start(
                    out_d[call * ROIS_PER_CALL:(call + 1) * ROIS_PER_CALL].rearrange(
                        "r (b c) -> b r c", c=C
                    ),
                    st[:].rearrange("b (r c) -> b r c", c=C),
                )
    nc.compile()
    return nc


def kernel(feat0, feat1, feat2, feat3, rois):
    from concourse.bass_utils import run_bass_kernel_spmd

    feats = [np.asarray(f, np.float32) for f in (feat0, feat1, feat2, feat3)]
    rois = np.asarray(rois, np.float32)
    # channels-last views per level/batch
    feats_T = [np.ascontiguousarray(f.transpose(0, 2, 3, 1)) for f in feats]
    metas = _roi_meta(rois)

    in_maps = []
    for core in range(N_CORES):
        core_rois = list(range(core * NROI_C, (core + 1) * NROI_C))
        win, idx_tiles, wmat = _build_core_inputs_fp16(feats_T, metas, core_rois)
        in_maps.append({"win": win, "idx": idx_tiles, "wts": wmat.reshape(NCALL * NSETS, 128, 4 * 49)})

    nc = _build_program()
    res = run_bass_kernel_spmd(nc, in_maps, core_ids=list(range(N_CORES)), trace=False)
    outs = []
    for core in range(N_CORES):
        o = res.results[core]["out"].astype(np.float32).reshape(NROI_C, 49, C)
        outs.append(np.ascontiguousarray(o.transpose(0, 2, 1)).reshape(NROI_C, C, OUT, OUT))
    return np.concatenate(outs, 0)


# Testing hook: emulate the device math in numpy (same win/idx/W data).
def emulate(feat0, feat1, feat2, feat3, rois):
    feats = [np.asarray(f, np.float32) for f in (feat0, feat1, feat2, feat3)]
    rois = np.asarray(rois, np.float32)
    feats_T = [np.ascontiguousarray(f.transpose(0, 2, 3, 1)) for f in feats]
    metas = _roi_meta(rois)
    out = np.zeros((N_ROIS, C, OUT, OUT), np.float32)
    for core in range(N_CORES):
        core_rois = list(range(core * NROI_C, (core + 1) * NROI_C))
        win, idx_tiles, wmat = _build_core_inputs(feats_T, metas, core_rois)
        winf = win.reshape(-1)
        for call in range(NCALL):
            idx_blk = idx_tiles[:16, call * (NI_CALL // 16):(call + 1) * (NI_CALL // 16)]
            slots = idx_blk.T.reshape(-1)
            base_off = call * ROIS_PER_CALL * WIN_STRIDE * (2 * C)
            G = np.zeros((NI_CALL, 4 * C), np.float32)
            for i in range(NREAL_CALL):
                st = base_off + int(slots[i]) * 2 * C
                G[i] = winf[st:st + 4 * C]
            accs = [np.zeros((49, C), np.float32) for _ in range(ROIS_PER_CALL)]
            for si_, (g_, j_) in enumerate(GROUP_SETS):
                K = GROUP_K[g_]
                W_ = wmat[call, si_]
                for q in range(4):
                    accs[j_] += W_[0:K, q * 49:(q + 1) * 49].T @ G[g_ * 128:g_ * 128 + K, q * C:(q + 1) * C]
            for j_ in range(ROIS_PER_CALL):
                r = core_rois[call * ROIS_PER_CALL + j_]
                out[r] = accs[j_].T.reshape(C, OUT, OUT)
    return out



# revision 2
# speedup vs baseline: 2.4183x; 2.4183x over previous
"""Multi-level (FPN) DeformRoIPool (zero-offset == aligned RoIAlign) for Trainium2.

Strategy (8 NeuronCores, SPMD, one Bass program):
- Host computes, per ROI, the set of DISTINCT feature pixels its 7x7x2x2
  bilinear sample grid touches (a Y x X grid product, ~0.37x the naive
  per-sample corner count) and lays them out channels-last as dense rows.
- ROIs are sorted by pixel count and dealt round-robin to the 8 cores so
  every core gets an identical slot-size profile -> one SPMD program.
- The bilinear+average reduction weight matrix factors as kron(Ay, Ax)
  ([nPix, 49] per ROI); host bakes it into per-128-row-group weight tiles.
- Device: big sequential HWDGE DMAs (no gather, no GpSimd), one matmul per
  (row-group, roi) set accumulating [49 bins, 256 ch] in PSUM, DVE cast to
  fp16, DMA out. Memory-bound by ~5-7 MB/core of HBM reads.
"""
import numpy as np

OUT = 7
SR = 2
STRIDES = (4, 8, 16, 32)
FINEST = 56.0
NLEV = 4
C = 256
NBIN = OUT * OUT
N_ROIS = 256
N_CORES = 8
NROI_C = N_ROIS // N_CORES          # 32 roi slots per core
FEAT_SHAPES = [(2, 256, 200, 200), (2, 256, 100, 100), (2, 256, 50, 50), (2, 256, 25, 25)]


# ---------------------------------------------------------------------------
# BIR fix: this container's walrus rejects >1 embedded sem wait per
# instruction (2 on EventSemaphore). Split excess waits onto EventSemaphore
# carriers at serialization time.
# ---------------------------------------------------------------------------
def _install_bir_waitsplit():
    import orjson
    import concourse.bass as bass

    if getattr(bass.Bass, "_waitsplit_patched", False):
        return

    def _fix_blocks(blocks, counter):
        for blk in blocks:
            insts = blk.get("instructions")
            if insts:
                out = []
                for ins in insts:
                    si = ins.get("sync_info")
                    ow = (si or {}).get("on_wait") or []
                    limit = 2 if ins.get("opcode") == "EventSemaphore" else 1
                    if len(ow) > limit:
                        excess = ow[: len(ow) - limit]
                        si["on_wait"] = ow[len(ow) - limit:]
                        for i in range(0, len(excess), 2):
                            counter[0] += 1
                            out.append({
                                "name": f"I-waitsplit-{counter[0]}",
                                "opcode": "EventSemaphore",
                                "engine": ins["engine"],
                                "ins": [], "outs": [],
                                "debug": ins.get("debug", 0),
                                "sync_info": {"on_update": [], "on_wait": excess[i:i + 2]},
                            })
                    out.append(ins)
                blk["instructions"] = out
            if blk.get("blocks"):
                _fix_blocks(blk["blocks"], counter)

    orig = bass.Bass.to_json_bytes

    def to_json_bytes(self, *a, **kw):
        data = orig(self, *a, **kw)
        d = orjson.loads(data)
        counter = [0]
        for fn in d.get("functions", []):
            _fix_blocks(fn.get("blocks", []), counter)
        return orjson.dumps(d) if counter[0] else data

    bass.Bass.to_json_bytes = to_json_bytes
    bass.Bass._waitsplit_patched = True


# ---------------------------------------------------------------------------
# Host-side layout computation
# ---------------------------------------------------------------------------
def _roi_meta(rois, feat_shapes):
    """Per-roi level + distinct pixel grid + separable weight factors.

    The reduction out[b=(i,jj), c] = sum_s w_s * F(sample corners)_c over the
    7x7x2x2 grid factors per ROI as kron(Ay, Ax): Ay[line, i], Ax[col, jj].
    """
    scale_wh = np.sqrt((rois[:, 3] - rois[:, 1]) * (rois[:, 4] - rois[:, 2]))
    with np.errstate(divide="ignore"):
        tl = np.clip(np.floor(np.log2(scale_wh / FINEST + 1e-6)), 0, NLEV - 1)
    tl = (tl + 1e-5).astype(np.int32)
    g = np.arange(OUT, dtype=np.float64)[:, None] + (np.arange(SR, dtype=np.float64)[None, :] + 0.5) / SR

    def axis_factor(lo, ext, L):
        """1D positions lo + ext/OUT * g -> (lines, A[nl, OUT]) weight factor."""
        p = lo + (ext / OUT) * g                    # [OUT, SR]
        v = (p > -1) & (p < L)
        pc = np.clip(p, 0.0, L - 1)
        p0 = np.minimum(np.floor(pc).astype(np.int64), L - 1)
        p1 = np.minimum(p0 + 1, L - 1)
        fr = pc - p0
        lines = np.unique(np.concatenate([p0.ravel(), p1.ravel()]))
        r0 = np.searchsorted(lines, p0)
        r1 = np.searchsorted(lines, p1)
        A = np.zeros((len(lines), OUT), np.float64)
        w0 = (1.0 - fr) * v / SR
        w1 = fr * v / SR
        for i in range(OUT):
            for s in range(SR):
                A[r0[i, s], i] += w0[i, s]
                A[r1[i, s], i] += w1[i, s]
        return lines, A

    metas = []
    for n in range(rois.shape[0]):
        l = int(tl[n])
        _, _, H, W = feat_shapes[l]
        sc = 1.0 / STRIDES[l]
        x1 = rois[n, 1] * sc - 0.5
        y1 = rois[n, 2] * sc - 0.5
        rw = rois[n, 3] * sc - 0.5 - x1
        rh = rois[n, 4] * sc - 0.5 - y1
        ylines, Ay = axis_factor(y1, rh, H)
        xlines, Ax = axis_factor(x1, rw, W)
        metas.append(dict(
            l=l, b=int(rois[n, 0]),
            ylines=ylines, xlines=xlines, Ay=Ay, Ax=Ax,
            npix=len(ylines) * len(xlines),
        ))
    return metas


def _build_layout(metas):
    """Uniform-across-cores slot layout.

    Sort rois by npix, deal round-robin: core c, slot k -> roi order[8k+c].
    Slot budget R_k = max npix over the octet -> identical structure per core.
    """
    order = np.argsort([m["npix"] for m in metas], kind="stable")
    budgets = []
    for k in range(NROI_C):
        octet = [metas[order[k * N_CORES + c]]["npix"] for c in range(N_CORES)]
        budgets.append(max(octet))
    offs = np.concatenate([[0], np.cumsum(budgets)]).astype(np.int64)
    r_total = int(offs[-1])
    ngrp = -(-r_total // 128)
    r_pad = ngrp * 128

    # sets: for each 128-row group, one matmul per slot whose budget range
    # intersects it. start/stop flag per slot's first/last set.
    sets = []           # (group, slot)
    first, last = {}, {}
    for gidx in range(ngrp):
        lo, hi = gidx * 128, gidx * 128 + 128
        for k in range(NROI_C):
            if offs[k] < hi and offs[k + 1] > lo:
                si = len(sets)
                sets.append((gidx, k))
                first.setdefault(k, si)
                last[k] = si
    return dict(order=order, budgets=budgets, offs=offs, r_total=r_total,
                ngrp=ngrp, r_pad=r_pad, sets=sets, first=first, last=last)


def _build_core_data(feats_T, metas, layout, core):
    """G rows + weight tiles for one core, in partition-major device layout."""
    ngrp, offs = layout["ngrp"], layout["offs"]
    sets = layout["sets"]
    G = np.zeros((layout["r_pad"], C), np.float32)
    W = np.zeros((len(sets), 128, NBIN), np.float32)
    for k in range(NROI_C):
        m = metas[layout["order"][k * N_CORES + core]]
        fT = feats_T[m["l"]][m["b"]]                     # [H, W, C]
        ny, nx = len(m["ylines"]), len(m["xlines"])
        G[offs[k]:offs[k] + ny * nx] = fT[m["ylines"]][:, m["xlines"]].reshape(-1, C)
    for si, (gidx, k) in enumerate(sets):
        m = metas[layout["order"][k * N_CORES + core]]
        ny, nx = len(m["ylines"]), len(m["xlines"])
        npix = ny * nx
        lo = max(gidx * 128, int(offs[k]))
        hi = min(gidx * 128 + 128, int(offs[k]) + npix)
        if hi <= lo:
            continue
        pix = np.arange(lo - offs[k], hi - offs[k])
        wy = m["Ay"][pix // nx]                          # [np, 7]
        wx = m["Ax"][pix % nx]                           # [np, 7]
        W[si, lo - gidx * 128:hi - gidx * 128] = np.einsum(
            "pi,pj->pij", wy, wx).reshape(-1, NBIN)
    # partition-major: row r -> [r % 128, r // 128]
    Gp = np.ascontiguousarray(
        G.reshape(ngrp, 128, C).transpose(1, 0, 2).reshape(128, ngrp * C)
    ).astype(np.float16)
    Wp = np.ascontiguousarray(
        W.transpose(1, 0, 2).reshape(128, len(sets) * NBIN)
    ).astype(np.float16)
    return Gp, Wp


# ---------------------------------------------------------------------------
# Device program
# ---------------------------------------------------------------------------
def _build_program(layout):
    import concourse.bacc as bacc
    import concourse.mybir as mybir
    import concourse.tile as tile

    _install_bir_waitsplit()
    nc = bacc.Bacc("TRN2", debug=False, enable_asserts=True, num_devices=N_CORES)

    ngrp = layout["ngrp"]
    sets = layout["sets"]
    nsets = len(sets)
    first, last = layout["first"], layout["last"]

    g_d = nc.dram_tensor("g", [128, ngrp * C], mybir.dt.float16, kind="ExternalInput")
    w_d = nc.dram_tensor("w", [128, nsets * NBIN], mybir.dt.float16, kind="ExternalInput")
    out_d = nc.dram_tensor("out", [NROI_C, NBIN * C], mybir.dt.float16, kind="ExternalOutput")

    # chunk the G stream on group boundaries for DMA/PE pipelining
    n_chunks = min(10, ngrp)
    bounds = [round(i * ngrp / n_chunks) for i in range(n_chunks + 1)]
    chunks = [(bounds[i], bounds[i + 1]) for i in range(n_chunks) if bounds[i + 1] > bounds[i]]

    with tile.TileContext(nc) as tc:
        with (
            tc.tile_pool(name="wp", bufs=1) as wp,
            tc.tile_pool(name="gp", bufs=3) as gp,
            tc.tile_pool(name="sp", bufs=1) as sp,
            tc.tile_pool(name="pp", bufs=8, space="PSUM") as pp,
        ):
            st = sp.tile([NBIN, NROI_C * C], mybir.dt.float16)
            wt = wp.tile([128, nsets * NBIN], mybir.dt.float16)
            ps_of = {}
            si = 0
            maxch = max(c1 - c0 for c0, c1 in chunks)
            for ci, (c0, c1) in enumerate(chunks):
                gt = gp.tile([128, maxch * C], mybir.dt.float16, tag="g")
                nc.sync.dma_start(gt[:, : (c1 - c0) * C], g_d[:, c0 * C:c1 * C])
                if ci == 0:
                    nc.sync.dma_start(wt[:], w_d[:])
                while si < len(sets) and sets[si][0] < c1:
                    gidx, k = sets[si]
                    if si == first[k]:
                        ps_of[k] = pp.tile([NBIN, C], mybir.dt.float32, tag="ps", name=f"ps_{k}")
                    nc.tensor.matmul(
                        out=ps_of[k][:, :],
                        lhsT=wt[:, si * NBIN:(si + 1) * NBIN],
                        rhs=gt[:, (gidx - c0) * C:(gidx - c0 + 1) * C],
                        start=(si == first[k]),
                        stop=(si == last[k]),
                    )
                    if si == last[k]:
                        nc.vector.tensor_copy(st[:, k * C:(k + 1) * C], ps_of[k][:])
                    si += 1
            for o0 in range(0, NROI_C, 8):
                nc.sync.dma_start(
                    out_d[o0:o0 + 8].rearrange("r (b c) -> b r c", c=C),
                    st[:, o0 * C:(o0 + 8) * C].rearrange("b (r c) -> b r c", c=C),
                )
    nc.compile()
    return nc


# ---------------------------------------------------------------------------
# Entry point
# ---------------------------------------------------------------------------
def kernel(feat0, feat1, feat2, feat3, rois):
    from concourse.bass_utils import run_bass_kernel_spmd

    feats = [np.asarray(f, np.float32) for f in (feat0, feat1, feat2, feat3)]
    rois = np.asarray(rois, np.float32)
    feat_shapes = [f.shape for f in feats]
    feats_T = [np.ascontiguousarray(f.transpose(0, 2, 3, 1)) for f in feats]
    metas = _roi_meta(rois, feat_shapes)
    layout = _build_layout(metas)

    in_maps = []
    for core in range(N_CORES):
        Gp, Wp = _build_core_data(feats_T, metas, layout, core)
        in_maps.append({"g": Gp, "w": Wp})

    nc = _build_program(layout)
    res = run_bass_kernel_spmd(nc, in_maps, core_ids=list(range(N_CORES)), trace=False)

    out = np.zeros((N_ROIS, C, OUT, OUT), np.float32)
    order = layout["order"]
    for core in range(N_CORES):
        o = res.results[core]["out"].astype(np.float32).reshape(NROI_C, NBIN, C)
        for k in range(NROI_C):
            out[order[k * N_CORES + core]] = o[k].T.reshape(C, OUT, OUT)
    return out


# Testing hook: emulate the device math in numpy (same G/W data).
def emulate(feat0, feat1, feat2, feat3, rois):
    feats = [np.asarray(f, np.float32) for f in (feat0, feat1, feat2, feat3)]
    rois = np.asarray(rois, np.float32)
    feat_shapes = [f.shape for f in feats]
    feats_T = [np.ascontiguousarray(f.transpose(0, 2, 3, 1)) for f in feats]
    metas = _roi_meta(rois, feat_shapes)
    layout = _build_layout(metas)
    sets = layout["sets"]
    out = np.zeros((N_ROIS, C, OUT, OUT), np.float32)
    for core in range(N_CORES):
        Gp, Wp = _build_core_data(feats_T, metas, layout, core)
        Gf = Gp.astype(np.float32).reshape(128, layout["ngrp"], C)
        Wf = Wp.astype(np.float32).reshape(128, len(sets), NBIN)
        acc = {k: np.zeros((NBIN, C), np.float32) for k in range(NROI_C)}
        for si, (gidx, k) in enumerate(sets):
            acc[k] += Wf[:, si, :].T @ Gf[:, gidx, :]
        for k in range(NROI_C):
            r = layout["order"][k * N_CORES + core]
            out[r] = acc[k].astype(np.float16).astype(np.float32).T.reshape(C, OUT, OUT)
    return out
